# revision 1
# baseline (speedup 1.0000x reference)
"""Causal multi-head self-attention (B=2, S=2048, D=1024, H=16) on 8 trn2 cores.

Sharding: tensor-parallel over heads — core c owns heads (2c, 2c+1), both
batches, full sequence. Per core: QKV projections for its 2 heads, RoPE,
causal attention, output-projection partial product; final sum over cores via
chunked ReduceScatter; host reassembles the full output.

All matmuls run as float32r (TF32-like, 1 cyc/row at N>=256), fp32 PSUM
accumulation. Softmax skips max-subtraction (scores ~ N(0,1), exp < 300).
Rowsums come free from an appended ones-column on V. Causal masking is done
post-exp with gpsimd affine_select on the diagonal blocks only.
"""
import math
import numpy as np

import concourse.bass as bass
from concourse import bacc
import concourse.mybir as mybir
from concourse.tile import TileContext
from concourse.bass_utils import run_bass_kernel_spmd

THETA = 10000.0
B, S, D, H = 2, 2048, 1024, 16
DH = D // H          # 64
NC = 8               # cores
HPC = H // NC        # heads per core = 2
R = B * S            # 4096 flat rows
SCALE = 1.0 / math.sqrt(DH)

f32 = mybir.dt.float32
f32r = mybir.dt.float32r

_CACHE = {}
import os
_DBG_NO_RS = bool(int(os.environ.get("DBG_NO_RS", "0")))
_DBG_STAGE = int(os.environ.get("DBG_STAGE", "4"))

_DBG_ATT = int(os.environ.get("DBG_ATT", "7"))  # bit0: affine, bit1: pv+norm, bit2: use partition_broadcast


def _build(_DBG_REPS=1):
    nc = bacc.Bacc(num_devices=NC)

    xt = nc.declare_dram_parameter("xt", [D, R], f32r, isOutput=False)
    wq = nc.declare_dram_parameter("wq", [D, 2 * DH], f32r, isOutput=False)
    wk = nc.declare_dram_parameter("wk", [D, 2 * DH], f32r, isOutput=False)
    wv = nc.declare_dram_parameter("wv", [D, 2 * DH], f32r, isOutput=False)
    wo = nc.declare_dram_parameter("wo", [2 * DH, D], f32r, isOutput=False)
    cost = nc.declare_dram_parameter("cost", [128, S], f32, isOutput=False)
    sint = nc.declare_dram_parameter("sint", [128, S], f32, isOutput=False)
    ones = nc.declare_dram_parameter("ones", [128, DH], f32r, isOutput=False)
    ident = nc.declare_dram_parameter("ident", [128, 128], f32, isOutput=False)
    yo = nc.declare_dram_parameter("yo", [4, 128, D], f32, isOutput=True)

    y_part = nc.dram_tensor("y_part", [R, D], f32)
    y_rs = nc.dram_tensor("y_rs", [4, 128, D], f32)

    NQ = 4            # xT column quarters in phase A
    QW = R // NQ      # 1024 rows per quarter
    NB = QW // 512    # 2 proj psum blocks per quarter

    with TileContext(nc) as tc:
        import contextlib
        ctx = contextlib.ExitStack()
        with ctx:
            # ---- persistent pools (whole kernel) ----
            pers = ctx.enter_context(tc.tile_pool(name="pers", bufs=1))
            exp_pool = ctx.enter_context(tc.tile_pool(name="expp", bufs=4))
            aux = ctx.enter_context(tc.tile_pool(name="aux", bufs=2))

            q_rope = pers.tile([128, R], f32r, name="q_rope")
            k_rope = pers.tile([128, R], f32r, name="k_rope")
            # V with ones column, natural rows layout: per batch [128, 16*130]
            v_sb = [pers.tile([128, (S // 128) * 130], f32r, name=f"v_sb{b}") for b in range(B)]
            attn = [pers.tile([128, S], f32r, name=f"attn{b}") for b in range(B)]
            wo_sb = pers.tile([128, D], f32r, name="wo_sb")
            ones_sb = pers.tile([128, DH], f32r, name="ones_sb")
            nc.sync.dma_start(out=wo_sb[:, :], in_=wo[:, :])
            nc.sync.dma_start(out=ones_sb[:, :], in_=ones[:, :])
            id_sb = pers.tile([128, 128], f32, name="id_sb")
            nc.sync.dma_start(out=id_sb[:, :], in_=ident[:, :])

            # ---- phase A: projections + RoPE + V assembly ----
            for _rep in range(_DBG_REPS):
              with tc.tile_pool(name="phA", bufs=1) as pha, \
                   tc.tile_pool(name="xtp", bufs=8) as xtp, \
                   tc.tile_pool(name="ropetmp", bufs=1) as rtp, \
                   tc.tile_pool(name="prj_ps", bufs=6, space="PSUM") as prj, \
                   tc.tile_pool(name="tp_ps", bufs=2, space="PSUM") as tpp:

                  wq_sb = pha.tile([128, 8, 2 * DH], f32r, name="wq_sb")
                  wk_sb = pha.tile([128, 8, 2 * DH], f32r, name="wk_sb")
                  wv_sb = pha.tile([128, 8, 2 * DH], f32r, name="wv_sb")
                  nc.sync.dma_start(out=wq_sb[:, :, :], in_=wq.rearrange("(t p) m -> p t m", p=128))
                  nc.sync.dma_start(out=wk_sb[:, :, :], in_=wk.rearrange("(t p) m -> p t m", p=128))
                  nc.sync.dma_start(out=wv_sb[:, :, :], in_=wv.rearrange("(t p) m -> p t m", p=128))
                  cos_sb = pha.tile([128, S], f32, name="cos_sb")
                  sin_sb = pha.tile([128, S], f32, name="sin_sb")
                  nc.sync.dma_start(out=cos_sb[:, :], in_=cost[:, :])
                  nc.sync.dma_start(out=sin_sb[:, :], in_=sint[:, :])

                  for qr in range(NQ):
                      c0 = qr * QW           # global row offset of this quarter
                      bq = c0 // S           # batch of this quarter
                      s0 = c0 % S            # seq offset of this quarter
                      xts = []
                      for k in range(8):
                          xk = xtp.tile([128, QW], f32r, name=f"xt{qr}_{k}", tag="xt")
                          nc.sync.dma_start(out=xk[:, :], in_=xt[k * 128:(k + 1) * 128, c0:c0 + QW])
                          xts.append(xk)

                      for tname, wsb, rope in (("q", wq_sb, q_rope), ("k", wk_sb, k_rope)):
                          pss = []
                          for n in range(NB):
                              ps = prj.tile([128, 512], f32, name=f"p{tname}{qr}{n}", tag="prj")
                              for k in range(8):
                                  nc.tensor.matmul(ps[:, :], wsb[:, k, :], xts[k][:, n * 512:(n + 1) * 512],
                                                   start=(k == 0), stop=(k == 7))
                              pss.append(ps)
                          # RoPE: partitions [0:64]=evens (h0e|h1e), [64:128]=odds
                          raw = rtp.tile([128, QW], f32, name=f"raw{tname}{qr}", tag="raw")
                          olo = rtp.tile([64, QW], f32, name=f"olo{tname}{qr}", tag="olo")
                          ehi = rtp.tile([128, QW], f32, name=f"ehi{tname}{qr}", tag="ehi")
                          p1 = rtp.tile([64, QW], f32, name=f"p1{tname}{qr}", tag="p1")
                          p3 = rtp.tile([128, QW], f32, name=f"p3{tname}{qr}", tag="p3")
                          for n in range(NB):
                              cs = slice(n * 512, (n + 1) * 512)
                              gs = slice(s0 + n * 512, s0 + (n + 1) * 512)
                              nc.scalar.copy(raw[:, cs], pss[n][:, :])
                              nc.vector.tensor_tensor(p1[0:64, cs], pss[n][0:64, :], cos_sb[0:64, gs], mybir.AluOpType.mult)
                              nc.vector.tensor_tensor(p3[64:128, cs], pss[n][64:128, :], cos_sb[64:128, gs], mybir.AluOpType.mult)
                          nc.sync.dma_start(out=olo[0:64, :], in_=raw[64:128, :])
                          nc.sync.dma_start(out=ehi[64:128, :], in_=raw[0:64, :])
                          gq = slice(s0, s0 + QW)
                          t2 = rtp.tile([64, QW], f32, name=f"t2{tname}{qr}", tag="t2")
                          t4 = rtp.tile([128, QW], f32, name=f"t4{tname}{qr}", tag="t4")
                          nc.vector.tensor_tensor(t2[0:64, :], olo[0:64, :], sin_sb[0:64, gq], mybir.AluOpType.mult)
                          nc.vector.tensor_tensor(t4[64:128, :], ehi[64:128, :], sin_sb[64:128, gq], mybir.AluOpType.mult)
                          ero = rtp.tile([64, QW], f32r, name=f"ero{tname}{qr}", tag="ero")
                          oro = rtp.tile([128, QW], f32r, name=f"oro{tname}{qr}", tag="oro")
                          nc.vector.tensor_tensor(ero[0:64, :], p1[0:64, :], t2[0:64, :], mybir.AluOpType.subtract)
                          nc.vector.tensor_tensor(oro[64:128, :], p3[64:128, :], t4[64:128, :], mybir.AluOpType.add)
                          # shuffle to head-contiguous: [h0e|h0o|h1e|h1o]
                          nc.sync.dma_start(out=rope[0:32, c0:c0 + QW], in_=ero[0:32, :])
                          nc.sync.dma_start(out=rope[64:96, c0:c0 + QW], in_=ero[32:64, :])
                          nc.sync.dma_start(out=rope[32:64, c0:c0 + QW], in_=oro[64:96, :])
                          nc.sync.dma_start(out=rope[96:128, c0:c0 + QW], in_=oro[96:128, :])

                      # V: transposed projection (N=512) then PE-transpose to natural
                      vt_sb = rtp.tile([128, QW], f32, name=f"vt{qr}", tag="vt")
                      for n in range(NB):
                          ps = prj.tile([128, 512], f32, name=f"pv{qr}{n}", tag="prj")
                          for k in range(8):
                              nc.tensor.matmul(ps[:, :], wv_sb[:, k, :], xts[k][:, n * 512:(n + 1) * 512],
                                               start=(k == 0), stop=(k == 7))
                          nc.vector.tensor_copy(vt_sb[:, n * 512:(n + 1) * 512], ps[:, :])
                      for rt in range(QW // 128):
                          gr = c0 + rt * 128                    # global row
                          sk = (gr % S) // 128                  # key tile within batch
                          vb = v_sb[gr // S]
                          tp = tpp.tile([128, 128], f32, name=f"tp{qr}{rt}", tag="tp")
                          nc.tensor.transpose(tp[:, :], vt_sb[:, rt * 128:(rt + 1) * 128], id_sb[:, :])
                          dst = vb[:, sk * 130: sk * 130 + 130].rearrange("p (h c) -> p h c", c=65)
                          src = tp[:, :].rearrange("p (h c) -> p h c", c=64)
                          nc.vector.tensor_copy(dst[:, :, 0:64], src[:, :, :])
                          nc.sync.dma_start(out=dst[:, :, 64:65],
                                            in_=ones_sb[:, 0:2].rearrange("p (h c) -> p h c", c=1))

              # ---- attention + output projection, chunked for RS overlap ----
              if _DBG_STAGE <= 1:
                  nc.sync.dma_start(out=y_part[0:128, :], in_=q_rope[:, 0:1024].bitcast(f32))
                  nc.sync.dma_start(out=y_part[128:256, :], in_=k_rope[:, 0:1024].bitcast(f32))
                  nc.sync.dma_start(out=y_part[256:384, :], in_=v_sb[0][:, 0:1024].bitcast(f32))
                  nc.sync.dma_start(out=y_rs[0], in_=y_part[0:128, :])
                  nc.sync.dma_start(out=yo[:, :, :], in_=y_rs[:, :, :])
              rctx = contextlib.ExitStack()
              sc_ps = rctx.enter_context(tc.tile_pool(name=f"sc_ps{_rep}", bufs=2, space="PSUM"))
              pv_ps = rctx.enter_context(tc.tile_pool(name=f"pv_ps{_rep}", bufs=2, space="PSUM"))
              ax_ps = rctx.enter_context(tc.tile_pool(name=f"ax_ps{_rep}", bufs=2, space="PSUM"))
              for b in range(B if _DBG_STAGE >= 2 else 0):
                  for qh in range(2):              # row-chunk of 1024 (4 q-blocks)
                      cc = b * 2 + qh
                      for h in range(HPC):
                          for qp in range(2):      # qb pair
                              qbs = (qh * 4 + qp * 2, qh * 4 + qp * 2 + 1)
                              pv = pv_ps.tile([65, 512], f32, name=f"pv{_rep}{cc}{h}{qp}", tag="pv")
                              for qi, qb in enumerate(qbs):
                                  nsk = 2 * (qb + 1)
                                  q_sl = slice(b * S + qb * 256, b * S + (qb + 1) * 256)
                                  for ch0 in range(0, nsk, 4):
                                      m = min(4, nsk - ch0)
                                      sc = sc_ps.tile([128, 1024], f32, name=f"sc{_rep}{cc}{h}{qp}{qi}{ch0}", tag="sc")
                                      for j in range(m):
                                          sk = ch0 + j
                                          k_sl = slice(b * S + sk * 128, b * S + (sk + 1) * 128)
                                          o = slice(j * 256, (j + 1) * 256)
                                          nc.tensor.matmul(sc[:, o], k_rope[64 * h:64 * h + 64, k_sl],
                                                           q_rope[64 * h:64 * h + 64, q_sl],
                                                           start=True, stop=True)
                                      ex = exp_pool.tile([128, 1024], f32r, name=f"ex{_rep}{cc}{h}{qp}{qi}{ch0}", tag="ex")
                                      nc.scalar.activation(ex[:, 0:m * 256], sc[:, 0:m * 256],
                                                           mybir.ActivationFunctionType.Exp, scale=SCALE)
                                      for j in range(m if (_DBG_ATT & 1) else 0):
                                          sk = ch0 + j
                                          o = slice(j * 256, (j + 1) * 256)
                                          if sk == 2 * qb:      # diagonal masking
                                              nc.gpsimd.affine_select(ex[:, o], ex[:, o], [[1, 256]],
                                                                      mybir.AluOpType.is_ge, 0.0,
                                                                      base=0, channel_multiplier=-1)
                                          elif sk == 2 * qb + 1:
                                              nc.gpsimd.affine_select(ex[:, o], ex[:, o], [[1, 256]],
                                                                      mybir.AluOpType.is_ge, 0.0,
                                                                      base=-128, channel_multiplier=-1)
                                      for j in range(m if (_DBG_ATT & 2) else 0):
                                          sk = ch0 + j
                                          o = slice(j * 256, (j + 1) * 256)
                                          nc.tensor.matmul(pv[:, qi * 256:(qi + 1) * 256],
                                                           v_sb[b][:, sk * 130 + 65 * h: sk * 130 + 65 * h + 65],
                                                           ex[:, o],
                                                           start=(sk == 0), stop=(sk == nsk - 1))
                              if not (_DBG_ATT & 2):
                                  a_sl = slice((qh * 2 + qp) * 512, (qh * 2 + qp + 1) * 512)
                                  nc.vector.tensor_copy(attn[b][:, a_sl], q_rope[:, b * S:(b + 1) * S][:, a_sl])
                                  continue
                              # normalize: out = pv[0:64] * (1/rowsum broadcast)
                              rec = aux.tile([1, 512], f32r, name=f"rec{_rep}{cc}{h}{qp}", tag="rec")
                              with nc.allow_low_precision(reason="softmax reciprocal"):
                                  nc.vector.reciprocal(rec[0:1, :], pv[64:65, :])
                              bc = aux.tile([64, 512], f32r, name=f"bc{_rep}{cc}{h}{qp}", tag="bc")
                              if _DBG_ATT & 4:
                                  nc.gpsimd.partition_broadcast(bc[0:64, :], rec[0:1, :], channels=64)
                              else:
                                  bcp = ax_ps.tile([128, 512], f32, name=f"bcp{_rep}{cc}{h}{qp}", tag="axp")
                                  nc.tensor.matmul(bcp[0:64, :], ones_sb[0:1, 0:64], rec[0:1, :],
                                                   start=True, stop=True)
                                  nc.vector.tensor_copy(bc[0:64, :], bcp[0:64, :])
                              a_sl = slice((qh * 2 + qp) * 512, (qh * 2 + qp + 1) * 512)
                              if h == 0:
                                  nc.vector.tensor_tensor(attn[b][0:64, a_sl], pv[0:64, :], bc[0:64, :],
                                                          mybir.AluOpType.mult)
                              else:
                                  hs = aux.tile([64, 512], f32r, name=f"hs{_rep}{cc}{qp}", tag="hs")
                                  nc.vector.tensor_tensor(hs[0:64, :], pv[0:64, :], bc[0:64, :],
                                                          mybir.AluOpType.mult)
                                  nc.sync.dma_start(out=attn[b][64:128, a_sl], in_=hs[0:64, :])
                      if _DBG_STAGE <= 2:
                          continue
                      # y chunk: rows b*S + qh*1024 .. +1024
                      for rt in range(8):
                          gr = b * S + qh * 1024 + rt * 128
                          for nb2 in range(2):
                              yp = ax_ps.tile([128, 512], f32, name=f"yp{_rep}{cc}{rt}{nb2}", tag="axp")
                              nc.tensor.matmul(yp[:, :], attn[b][:, (gr % S):(gr % S) + 128],
                                               wo_sb[:, nb2 * 512:(nb2 + 1) * 512],
                                               start=True, stop=True)
                              yc = aux.tile([128, 512], f32, name=f"yc{_rep}{cc}{rt}{nb2}", tag="yc", bufs=3)
                              if (rt + nb2) % 2 == 0:
                                  nc.scalar.copy(yc[:, :], yp[:, :])
                              else:
                                  nc.vector.tensor_copy(yc[:, :], yp[:, :])
                              nc.sync.dma_start(out=y_part[gr:gr + 128, nb2 * 512:(nb2 + 1) * 512],
                                                in_=yc[:, :])
                      if not _DBG_NO_RS:
                          nc.gpsimd.collective_compute(
                              "ReduceScatter", mybir.AluOpType.add,
                              replica_groups=[list(range(NC))],
                              ins=[y_part[cc * 1024:(cc + 1) * 1024, :]],
                              outs=[y_rs[cc]],
                          )
                      else:
                          nc.sync.dma_start(out=y_rs[cc], in_=y_part[cc * 1024:cc * 1024 + 128, :])
              rctx.close()
            if _DBG_STAGE == 2:
                nc.sync.dma_start(out=y_part[0:128, :], in_=attn[0][:, 0:1024].bitcast(f32))
                nc.sync.dma_start(out=y_rs[0], in_=y_part[0:128, :])
            if _DBG_STAGE >= 2:
                nc.sync.dma_start(out=yo[:, :, :], in_=y_rs[:, :, :])
    nc.finalize()
    return nc


def _host_inputs(in_features, token_positions, Wq, Wk, Wv, Wo):
    x = np.ascontiguousarray(in_features, dtype=np.float32).reshape(R, D)
    xt = np.ascontiguousarray(x.T)
    pos = np.asarray(token_positions, dtype=np.float64)
    inv = THETA ** (-np.arange(0, DH, 2, dtype=np.float64) / DH)   # [32]
    ang = pos[None, :] * inv[:, None]                              # [32, S]
    cos32 = np.cos(ang).astype(np.float32)
    sin32 = np.sin(ang).astype(np.float32)
    cost = np.tile(cos32, (4, 1))                                  # [128, S]
    sint = np.tile(sin32, (4, 1))
    ones = np.ones((128, DH), dtype=np.float32)
    ident = np.eye(128, dtype=np.float32)

    in_maps = []
    for c in range(NC):
        h0 = HPC * c
        rows = []
        for j in range(HPC):
            rows += [(h0 + j) * DH + 2 * i for i in range(DH // 2)]      # evens
        for j in range(HPC):
            rows += [(h0 + j) * DH + 2 * i + 1 for i in range(DH // 2)]  # odds
        wq_c = np.ascontiguousarray(Wq[rows, :].T, dtype=np.float32)     # [D, 128]
        wk_c = np.ascontiguousarray(Wk[rows, :].T, dtype=np.float32)
        vrows = list(range(h0 * DH, (h0 + HPC) * DH))
        wv_c = np.ascontiguousarray(Wv[vrows, :].T, dtype=np.float32)    # [D, 128]
        wo_c = np.ascontiguousarray(Wo[:, vrows].T, dtype=np.float32)    # [128, D]
        in_maps.append({
            "xt": xt, "wq": wq_c, "wk": wk_c, "wv": wv_c, "wo": wo_c,
            "cost": cost, "sint": sint, "ones": ones, "ident": ident,
        })
    return in_maps


def _assemble(results):
    y = np.empty((R, D), dtype=np.float32)
    for c in range(NC):
        pieces = results[c]["yo"]          # [4, 128, D] — rank c's RS shard per chunk
        for cc in range(4):
            r0 = cc * 1024 + c * 128
            y[r0:r0 + 128, :] = pieces[cc]
    return y.reshape(B, S, D)


def kernel(in_features, token_positions, Wq, Wk, Wv, Wo):
    if "nc" not in _CACHE:
        _CACHE["nc"] = _build()
    nc = _CACHE["nc"]
    in_maps = _host_inputs(in_features, token_positions, Wq, Wk, Wv, Wo)
    res = run_bass_kernel_spmd(nc, in_maps, list(range(NC)))
    return _assemble(res.results)

